# revision 2
# baseline (speedup 1.0000x reference)
"""PoolHiddenNet-style GNN message passing kernel for 8 Trainium2 cores.

Math (per group s of S=32, P=64 peds, uniform groups):
  rel[i,j]  = obs[j] - obs[i]                         (P^2, 16)
  emb       = rel @ W_sp + b_sp                       (P^2, 512)
  x_a       = tw * emb          tw[n, t*64+k] = twq[n, t*2+k%2]
  x1        = relu(bn([x_a, h1] @ W1 + b1))           (P^2, 512)
  x2        = relu(bn(x1 @ W2 + b2))                  (P^2, 1024)
  out       = max over j                              (P, 1024)

Key transforms used here:
  * b1/b2 cancel inside train-mode BN (bias shifts the mean equally).
  * tw*emb @ W1a == z @ C with z[n, q*16+r] = twq[n,q]*rel[n,r] and
    C[q*16+r, d] = sum_{f: q(f)=q} W_sp[r,f] W1a[f,d]  (K 576 -> 336).
    b_sp contributes twq @ Cb with Cb[q,d] = sum_{f:q(f)=q} b_sp[f] W1a[f,d].
  * rel and the i-replicated h1 are precomputed on the host (pure input
    preprocessing), so the kernel's z-operand DMA expansions read
    straight from DRAM inputs and the k3 h-part is a plain copy.
  * h1 @ W1b uses h1T = hT broadcast over i (only 64 distinct rows).
  * BN2 apply is monotone (gamma*rsqrt > 0), so max-pool first, then
    apply BN+relu on the pooled (P, 1024) values only.
  * x2 is never evicted: max-pool stage 1 (j 64->32, TT-max) reads PSUM
    directly and writes bf16; stages 2-6 run in bf16 2x mode.  Var2 is
    a per-px sum-of-squares, split between ACT (Square+accum_out) and
    DVE (tensor_tensor_reduce) by the ACT_SQ_PX knob; mean2 comes from
    colsum(x1n) (BN1-apply accum_out) via thin matmuls on the PE.
  * The issue order interleaves x1(g+1) and x2(g) dch-by-dch so the
    strict-FIFO ACT/DVE queues never head-of-line-block the shared
    PSUM pool; BN2-finalize runs per-dch, one slot behind its dch.
  * z DMAs issue at the iteration top but their GP multiplies at the
    bottom, so the GP queue serves BN2 applies promptly.
  * Everything runs feature-on-partition (transposed activations); the
    output leaves via DVE 32x32 block-transpose + a block-stitching
    DMA access pattern, so no PE instruction depends on the BN2 chain.

Sharding: data-parallel over S; core c handles groups 4c..4c+3.
"""

import os
import numpy as np
import ml_dtypes

S, P = 32, 64
PP = P * P                  # 4096
OBS, EMB, HDIM = 8, 64, 64
D1, D2 = 512, 1024
NCORES = 8
G = S // NCORES             # 4 groups per core
EPS = 1e-5

BF16 = np.float16
# matmul/operand dtype for the main chain ("bf16" or "f32")
MM_DTYPE = os.environ.get("KERNEL_MM_DTYPE", "f16")
# of the 32 x2 psum tiles per group, how many get their sum-of-squares
# via ACT Square+accum (the rest use DVE tensor_tensor_reduce)
ACT_SQ_PX = int(os.environ.get("KERNEL_ACT_SQ_PX", "26"))

_PROG_CACHE = {}
LAST_RESULTS = None


def _np_mm_dtype():
    return np.float32 if MM_DTYPE == "f32" else BF16


def build_program():
    """Build (and compile) the per-core Bass program. Returns nc."""
    import concourse.bacc as bacc
    import concourse.mybir as mybir
    import concourse.tile as tile
    from concourse import masks

    f32 = mybir.dt.float32
    DT = mybir.dt.float32 if MM_DTYPE == "f32" else mybir.dt.float16
    AF = mybir.ActivationFunctionType
    ALU = mybir.AluOpType

    nc = bacc.Bacc("TRN2", target_bir_lowering=False, debug=False)

    # ---- DRAM I/O ----
    d_reld = nc.dram_tensor("reld", [G * 16, PP], DT, kind="ExternalInput")
    d_twqT = nc.dram_tensor("twqT", [16, G, PP], DT, kind="ExternalInput")
    d_h1r = nc.dram_tensor("h1rep", [HDIM, G, PP], DT, kind="ExternalInput")
    d_C = nc.dram_tensor("Csb", [128, 2, D1], DT, kind="ExternalInput")
    d_CbW = nc.dram_tensor("CbW", [16 + HDIM, D1], DT, kind="ExternalInput")
    d_W2 = nc.dram_tensor("W2sb", [128, 4, D2], DT, kind="ExternalInput")
    d_g1 = nc.dram_tensor("g1c", [128, 4], f32, kind="ExternalInput")
    d_be1 = nc.dram_tensor("be1c", [128, 4], f32, kind="ExternalInput")
    d_g2 = nc.dram_tensor("g2c", [128, 8], f32, kind="ExternalInput")
    d_be2 = nc.dram_tensor("be2c", [128, 8], f32, kind="ExternalInput")
    d_out = nc.dram_tensor("out", [G * P, D2], f32, kind="ExternalOutput")

    HF = PP // 2

    with tile.TileContext(nc) as tc:
        with (
            tc.tile_pool(name="singles", bufs=1) as singles,
            tc.tile_pool(name="work", bufs=2) as work,
            tc.tile_pool(name="stat", bufs=2) as stat,
            tc.tile_pool(name="psmm", bufs=4, space="PSUM") as psmm,
            tc.tile_pool(name="dscr", bufs=2, space="DRAM") as dscr,
        ):
            Csb = singles.tile([128, 2, D1], DT)
            CbW = singles.tile([16 + HDIM, D1], DT)
            W2sb = singles.tile([128, 4, D2], DT)
            g1c = singles.tile([128, 4], f32)
            be1c = singles.tile([128, 4], f32)
            g2c = singles.tile([128, 8], f32)
            be2c = singles.tile([128, 8], f32)
            eps_t = singles.tile([128, 1], f32)
            ident = singles.tile([128, 128], f32)

            n_groups = int(os.environ.get("KERNEL_GROUPS", G))

            def z_dma(g, k3_eng=None):
                """z operand DMA-expansion + k3 loads (h pre-replicated
                on the host so the k3 DMA is plain-copy, not broadcast).
                k3_eng selects the DMA queue for the k3 loads (the
                prologue uses the ACT queue to halve startup latency)."""
                k3_eng = k3_eng or nc.sync
                zT = work.tile([128, 2, PP], DT, tag="zT")
                k3 = work.tile([16 + HDIM, PP], DT, tag="k3")
                abs_ = []
                for h in range(2):
                    B_h = work.tile([128, HF], DT, tag="bsb")
                    nc.sync.dma_start(
                        out=B_h[:],
                        in_=d_reld.ap()[None, g * 16:g * 16 + 16,
                                        h * HF:(h + 1) * HF]
                        .broadcast_to((8, 16, HF)))
                    As = []
                    for kc in range(2):
                        A_h = work.tile([128, HF], DT, tag="asb")
                        nc.sync.dma_start(
                            out=A_h[:],
                            in_=d_twqT.ap()[8 * kc:8 * kc + 8, g, None,
                                            h * HF:(h + 1) * HF]
                            .broadcast_to((8, 16, HF)))
                        As.append(A_h)
                    abs_.append((B_h, As))
                    k3_eng.dma_start(out=k3[0:16, h * HF:(h + 1) * HF],
                                     in_=d_twqT.ap()[:, g,
                                                     h * HF:(h + 1) * HF])
                    k3_eng.dma_start(
                        out=k3[16:, h * HF:(h + 1) * HF],
                        in_=d_h1r.ap()[:, g, h * HF:(h + 1) * HF])
                return zT, k3, abs_

            def z_mult(zk):
                """zT[q*16+r, n] = twqT[q, n] * relT[r, n] on GpSimd,
                in quarter-chunks so the first x1 matmul columns unblock
                as soon as the corresponding DMA quarter lands."""
                zT, k3, abs_ = zk
                Q = HF // 2
                for h in range(2):
                    B_h, As = abs_[h]
                    for q in range(2):
                        for kc in range(2):
                            nc.gpsimd.tensor_tensor(
                                out=zT[:, kc, h * HF + q * Q:
                                       h * HF + (q + 1) * Q],
                                in0=As[kc][:, q * Q:(q + 1) * Q],
                                in1=B_h[:, q * Q:(q + 1) * Q], op=ALU.mult)

            # prologue: first z-operands before the big weight loads (SP
            # queue); weights go on the ACT hwdge queue in parallel.
            for t_sb, t_dr in [
                (Csb, d_C), (CbW, d_CbW), (g1c, d_g1), (be1c, d_be1),
                (g2c, d_g2), (be2c, d_be2),
            ]:
                nc.scalar.dma_start(out=t_sb[:], in_=t_dr.ap())
            zks = [z_dma(0)]
            nc.vector.memset(eps_t[:], EPS)
            masks.make_identity(nc, ident[:])
            z_mult(zks[0])
            if n_groups > 1:
                zks.append(z_dma(1))
            # W2sb (1 MB, first needed by x2(0) / mean2 ~45us in) loads on
            # the SP queue behind the z expansions, keeping the ACT queue
            # clear for the first x1 evicts.
            nc.sync.dma_start(out=W2sb[:], in_=d_W2.ap())
            if n_groups > 1:
                z_mult(zks[1])

            def x1_dch(zT, k3, x1, s1np, dch, psum_stats=False,
                       apply_prio=-150):
                """One dch (128 feats) of x1: 4 psum tiles of matmuls,
                ACT-evict to SBUF bf16, DVE bn_stats (from the SBUF copy,
                or straight from PSUM when psum_stats -- used for group 0
                to shorten the startup stats->apply cascade), BN1 chain,
                fused ACT apply+relu (accum->s1n)."""
                d0 = dch * 128
                stats1 = stat.tile([128, 8, 6], f32, tag="stats1")
                for nc2 in range(4):
                    px = psmm.tile([128, 2, 512], f32, tag="mm")
                    # kc-outer so consecutive matmuls share the lhsT
                    for nh in range(2):
                        n0 = nc2 * 1024 + nh * 512
                        nc.tensor.matmul(px[:, nh, :],
                                         Csb[:, 0, d0:d0 + 128],
                                         zT[:, 0, n0:n0 + 512],
                                         start=True, stop=False)
                    for nh in range(2):
                        n0 = nc2 * 1024 + nh * 512
                        nc.tensor.matmul(px[:, nh, :],
                                         Csb[:, 1, d0:d0 + 128],
                                         zT[:, 1, n0:n0 + 512],
                                         start=False, stop=False)
                    for nh in range(2):
                        n0 = nc2 * 1024 + nh * 512
                        nc.tensor.matmul(px[:, nh, :],
                                         CbW[:, d0:d0 + 128],
                                         k3[:, n0:n0 + 512],
                                         start=False, stop=True)
                    if psum_stats:
                        for nh in range(2):
                            nc.vector.bn_stats(
                                out=stats1[:, nc2 * 2 + nh, :],
                                in_=px[:, nh, :])
                    nc.scalar.activation(
                        out=x1[:, dch, nc2 * 1024:(nc2 + 1) * 1024],
                        in_=px[:].rearrange("p a b -> p (a b)"),
                        func=AF.Copy)
                if not psum_stats:
                    for c in range(8):
                        nc.vector.bn_stats(
                            out=stats1[:, c, :],
                            in_=x1[:, dch, c * 512:(c + 1) * 512])
                mv1 = stat.tile([128, 2], f32, tag="mv1")
                nc.vector.bn_aggr(out=mv1[:], in_=stats1[:])
                std1 = stat.tile([128, 1], f32, tag="std1")
                gam1 = stat.tile([128, 1], f32, tag="gam1")
                bet1 = stat.tile([128, 1], f32, tag="bet1")
                nc.scalar.activation(out=std1[:], in_=mv1[:, 1:2],
                                     func=AF.Sqrt, bias=eps_t[:])
                nc.vector.reciprocal(out=std1[:], in_=std1[:])
                nc.vector.tensor_mul(gam1[:], g1c[:, dch:dch + 1], std1[:])
                nc.vector.tensor_mul(bet1[:], mv1[:, 0:1], gam1[:])
                nc.vector.tensor_sub(bet1[:], be1c[:, dch:dch + 1], bet1[:])
                # the apply's consumer (x2 of the NEXT group) is a full
                # iteration away; deprioritize it AND chunk it so the
                # px-draining Squares of the current x2 dch are never stuck
                # behind a 3.8us monolith on the strict-FIFO ACT queue
                with tc.high_priority(offset=apply_prio):
                    for c in range(4):
                        nc.scalar.activation(
                            out=x1[:, dch, c * 1024:(c + 1) * 1024],
                            in_=x1[:, dch, c * 1024:(c + 1) * 1024],
                            func=AF.Relu, bias=bet1[:], scale=gam1[:],
                            accum_out=s1np[:, dch, c, None])

            def mean2_start(s1np):
                """mean2 (transposed, [1, 1024]) via thin matmuls, then
                redistributed to [128, 8] through a DRAM scratch bounce."""
                s1n = stat.tile([128, 4], f32, tag="s1nr")
                nc.vector.reduce_sum(s1n[:], s1np[:],
                                     axis=mybir.AxisListType.X)
                s1nd = stat.tile([128, 4], DT, tag="s1nd")
                nc.vector.tensor_copy(s1nd[:], s1n[:])
                pm2 = psmm.tile([1, 2, 512], f32, tag="mm")
                for kc in range(4):
                    for hh in range(2):
                        nc.tensor.matmul(
                            pm2[:, hh, :], s1nd[:, kc:kc + 1],
                            W2sb[:, kc, hh * 512:(hh + 1) * 512],
                            start=(kc == 0), stop=(kc == 3))
                sum2 = stat.tile([1, 1024], f32, tag="sum2")
                nc.scalar.mul(out=sum2[:], in_=pm2[:].rearrange(
                    "p a b -> p (a b)"), mul=1.0 / PP)
                m2d = dscr.tile([1, 1024], f32, tag="m2d")
                nc.sync.dma_start(out=m2d[:], in_=sum2[:])
                mean2 = stat.tile([128, 8], f32, tag="mean2")
                nc.sync.dma_start(
                    out=mean2[:],
                    in_=m2d[:].rearrange("p (a b) -> (p b) a", a=8))
                return mean2

            def x2_core(g, x1, dch):
                """One dch (128 feats) of x2: 4 psum tiles of matmuls;
                per tile ACT Square+accum (sumsq) and DVE reduce_max over
                j (both single-PSUM-operand; they drain different tiles
                concurrently since the banks differ)."""
                d0 = dch * 128
                ssqd = stat.tile([128, 4], f32, tag="ssqd", bufs=4)
                pooled = stat.tile([128, P], f32, tag="pooled", bufs=8)
                for nc2 in range(4):
                    px = psmm.tile([128, 2, 512], f32, tag="mm")
                    # kc-outer so consecutive matmuls share the lhsT
                    for kc in range(4):
                        for nh in range(2):
                            n0 = nc2 * 1024 + nh * 512
                            nc.tensor.matmul(
                                px[:, nh, :], W2sb[:, kc, d0:d0 + 128],
                                x1[:, kc, n0:n0 + 512],
                                start=(kc == 0), stop=(kc == 3))
                    pxf = px[:].rearrange("p a b -> p (a b)")
                    sqj = work.tile([128, 1024], DT, tag="sqj", bufs=4)
                    nc.scalar.activation(
                        out=sqj[:], in_=pxf, func=AF.Square,
                        accum_out=ssqd[:, nc2:nc2 + 1])
                    nc.vector.reduce_max(
                        pooled[:, nc2 * 16:(nc2 + 1) * 16],
                        pxf.rearrange("p (i j) -> p i j", j=P),
                        axis=mybir.AxisListType.X)
                ssqt = stat.tile([128, 1], f32, tag="ssqt", bufs=4)
                nc.vector.reduce_sum(ssqt[:], ssqd[:],
                                     axis=mybir.AxisListType.X)
                return ssqt, pooled

            def x2_fin(g, mean2, gb, core_ctx, dch):
                """Per-dch BN2 finalize: var2 = sumsq/N - mean2^2; the
                gamma'/beta' land in the packed gb tile for out_half."""
                ssqt, _pooled = core_ctx
                m2 = mean2[:, dch:dch + 1]
                m2sq = stat.tile([128, 1], f32, tag="m2sq")
                nc.vector.tensor_mul(m2sq[:], m2, m2)
                var2 = stat.tile([128, 1], f32, tag="var2")
                nc.vector.scalar_tensor_tensor(
                    out=var2[:], in0=ssqt[:], scalar=1.0 / PP, in1=m2sq[:],
                    op0=ALU.mult, op1=ALU.subtract)
                std2 = stat.tile([128, 1], f32, tag="std2")
                gam2 = gb[:, 0, dch % 4, None]
                bet2 = gb[:, 1, dch % 4, None]
                nc.scalar.activation(out=std2[:], in_=var2[:],
                                     func=AF.Sqrt, bias=eps_t[:])
                nc.vector.reciprocal(out=std2[:], in_=std2[:])
                nc.vector.tensor_mul(gam2, g2c[:, dch:dch + 1], std2[:])
                nc.vector.tensor_mul(bet2, m2, gam2)
                nc.vector.tensor_sub(bet2, be2c[:, dch:dch + 1], bet2)

            def out_half(g, pools, gb, q4):
                """Batched PE transposes of the RAW pooled tiles (dep =
                max-tree only), then BN2 apply + relu in row-major layout
                on GpSimd: gam/bet get partition-broadcast via a DRAM
                bounce; one plain row-major DMA writes the half."""
                pst = psmm.tile([P, 4, 128], f32, tag="mm")
                for i, pl in enumerate(pools):
                    nc.tensor.transpose(pst[:, i, :], pl[:], ident[:])
                rows = stat.tile([P, 4, 128], f32, tag="rows")
                nc.vector.tensor_copy(rows[:], pst[:])
                gbd = dscr.tile([8, 128], f32, tag="gbd")
                nc.sync.dma_start(
                    out=gbd[:].rearrange("s f -> f s"),
                    in_=gb[:].rearrange("f s d -> f (s d)"))
                gbr = stat.tile([P, 8, 128], f32, tag="gbr")
                nc.sync.dma_start(
                    out=gbr[:],
                    in_=gbd[:].rearrange("s f -> (s f)")[None, :]
                    .broadcast_to((P, 1024)))
                nc.gpsimd.tensor_tensor(
                    out=rows[:], in0=rows[:], in1=gbr[:, 0:4, :],
                    op=ALU.mult)
                nc.gpsimd.tensor_tensor(
                    out=rows[:], in0=rows[:], in1=gbr[:, 4:8, :],
                    op=ALU.add)
                rws = rows[:].rearrange("p a b -> p (a b)")
                nc.gpsimd.tensor_relu(rws, rws)
                nc.sync.dma_start(
                    out=d_out.ap()[g * P:(g + 1) * P,
                                   q4 * 512:(q4 + 1) * 512],
                    in_=rws)

            def out_last(g, pools, gb):
                """Last half of the last group: feature-major GP apply +
                PE transposes (PE is drained by now), one contiguous DMA."""
                pq = stat.tile([128, 4, P], f32, tag="pqlast")
                for i, pl in enumerate(pools):
                    nc.gpsimd.tensor_scalar(
                        out=pq[:, i], in0=pl[:],
                        scalar1=gb[:, 0, i, None], scalar2=gb[:, 1, i, None],
                        op0=ALU.mult, op1=ALU.add)
                    nc.gpsimd.tensor_relu(pq[:, i], pq[:, i])
                pst = psmm.tile([P, 4, 128], f32, tag="mm")
                for i in range(4):
                    nc.tensor.transpose(pst[:, i, :], pq[:, i], ident[:])
                out_rows = stat.tile([P, 4, 128], f32, tag="rows")
                nc.vector.tensor_copy(out_rows[:], pst[:])
                nc.sync.dma_start(
                    out=d_out.ap()[g * P:(g + 1) * P, 512:1024],
                    in_=out_rows[:].rearrange("p a b -> p (a b)"))

            def x1_alloc():
                x1 = work.tile([128, 4, PP], DT, tag="x1")
                s1np = stat.tile([128, 4, 4], f32, tag="s1n")
                return x1, s1np

            # software pipeline: z(g+2) DMA prefetch at iteration top, its
            # GP mults at the bottom (so GP applies aren't FIFO-blocked);
            # x1(g+1) dchs interleaved with x2(g) dchs; x2 finalize lags
            # its core by one slot; mean2 at the top of the iteration
            # (k=1 of iteration 0, past the startup ACT cascade).
            x1s = x1_alloc()
            for dch in range(4):
                x1_dch(zks[0][0], zks[0][1], *x1s, dch)
            for g in range(n_groups):
                x1, s1n = x1s
                mean2 = None
                if g + 1 < n_groups:
                    x1s = x1_alloc()
                cores = {}
                fctx = {}
                gb = stat.tile([128, 2, 4], f32, tag="gb")
                for k in range(4):
                    def x1_slot():
                        if g + 1 < n_groups:
                            x1_dch(zks[g + 1][0], zks[g + 1][1], *x1s, k)
                    if g > 0:
                        x1_slot()
                    if k == 2:
                        gb_lo, gb = gb, stat.tile([128, 2, 4], f32,
                                                  tag="gb")
                    cores[2 * k] = x2_core(g, x1, 2 * k)
                    if k == 1:
                        mean2 = mean2_start(s1n)
                    if k == 1 and g + 2 < n_groups:
                        zks.append(z_dma(g + 2))
                    cores[2 * k + 1] = x2_core(g, x1, 2 * k + 1)
                    if g == 0:
                        x1_slot()
                    if k >= 1:
                        fin_gb = gb_lo if k == 2 else gb
                        for d in (2 * k - 2, 2 * k - 1):
                            fctx[d] = cores.pop(d)
                            x2_fin(g, mean2, fin_gb, fctx[d], d)
                        if k == 2:
                            out_half(g, [fctx[d][1] for d in range(4)],
                                     gb_lo, 0)
                for dch in range(6, 8):
                    fctx[dch] = cores.pop(dch)
                    x2_fin(g, mean2, gb, fctx[dch], dch)
                if g == n_groups - 1:
                    out_last(g, [fctx[d][1] for d in range(4, 8)], gb)
                else:
                    out_half(g, [fctx[d][1] for d in range(4, 8)], gb, 1)
                if g + 2 < n_groups:
                    z_mult(zks[g + 2])

    nc.compile()
    return nc


def _host_prepare(inputs):
    """Slice/permute full inputs into 8 per-core in_maps (host-side)."""
    dtm = _np_mm_dtype()
    f32 = np.float32

    h_states = np.asarray(inputs["h_states"], f32)
    traj = np.asarray(inputs["traj"], f32)
    traj_weight = np.asarray(inputs["traj_weight"], f32)
    W_sp = np.asarray(inputs["W_sp"], f32)
    b_sp = np.asarray(inputs["b_sp"], f32)
    W1 = np.asarray(inputs["W1"], f32)
    g1 = np.asarray(inputs["g1"], f32)
    be1 = np.asarray(inputs["be1"], f32)
    W2 = np.asarray(inputs["W2"], f32)
    g2 = np.asarray(inputs["g2"], f32)
    be2 = np.asarray(inputs["be2"], f32)

    # obs: (S, P, 16) with feature index t*2+c
    obs = np.transpose(traj[:OBS], (1, 0, 2)).reshape(S, P, OBS * 2)
    h = h_states.reshape(S, P, HDIM)

    # relT[s, r, i*64+j] = obs[s, j, r] - obs[s, i, r]
    obsT = obs.transpose(0, 2, 1)                      # (S, 16, P)
    relT = (obsT[:, :, None, :] - obsT[:, :, :, None]).reshape(S, 16, PP)

    # C fold: q(f) = (f//64)*2 + f%2
    f_idx = np.arange(EMB * OBS)
    qof = (f_idx // EMB) * 2 + (f_idx % 2)
    W1a, W1b = W1[:D1], W1[D1:]
    C = np.zeros((256, D1), f32)
    Cb = np.zeros((16, D1), f32)
    for q in range(16):
        m = qof == q
        C[q * 16:(q + 1) * 16] = W_sp[:, m] @ W1a[m]
        Cb[q] = b_sp[m] @ W1a[m]
    Csb = np.ascontiguousarray(C.reshape(2, 128, D1).transpose(1, 0, 2))
    W2sb = np.ascontiguousarray(W2.reshape(4, 128, D2).transpose(1, 0, 2))

    shared = {
        "Csb": Csb.astype(dtm),
        "CbW": np.concatenate([Cb, W1b], axis=0).astype(dtm),
        "W2sb": W2sb.astype(dtm),
        "g1c": np.ascontiguousarray(g1.reshape(4, 128).T),
        "be1c": np.ascontiguousarray(be1.reshape(4, 128).T),
        "g2c": np.ascontiguousarray(g2.reshape(8, 128).T),
        "be2c": np.ascontiguousarray(be2.reshape(8, 128).T),
    }

    in_maps = []
    for c in range(NCORES):
        sl = slice(c * G, (c + 1) * G)
        reld = np.ascontiguousarray(relT[sl].reshape(G * 16, PP))
        twqT = np.ascontiguousarray(
            traj_weight[sl].transpose(3, 2, 0, 1).reshape(16, G, PP))
        hT = h[sl].transpose(2, 0, 1)                                 # (64,G,P)
        h1rep = np.broadcast_to(hT[:, :, None, :],
                                (HDIM, G, P, P)).reshape(HDIM, G, PP)
        in_maps.append({
            "reld": reld.astype(dtm),
            "twqT": twqT.astype(dtm),
            "h1rep": np.ascontiguousarray(h1rep).astype(dtm),
            **shared,
        })
    return in_maps


def kernel(**inputs) -> np.ndarray:
    global LAST_RESULTS
    from concourse import bass_utils

    if "prog" not in _PROG_CACHE:
        _PROG_CACHE["prog"] = build_program()
    nc = _PROG_CACHE["prog"]

    in_maps = _host_prepare(inputs)
    trace = bool(int(os.environ.get("KERNEL_TRACE", "0")))
    res = bass_utils.run_bass_kernel_spmd(
        nc, in_maps, core_ids=list(range(NCORES)), trace=trace)
    LAST_RESULTS = res
    out = np.concatenate([res.results[c]["out"] for c in range(NCORES)], axis=0)
    return out.astype(np.float32)

